# revision 37
# baseline (speedup 1.0000x reference)
"""DeMBR multi-behavior LightGCN kernel for Trainium2 (8 NeuronCores).

Strategy (per dense behavior, each [N,N] relation matrix R):
  - Host pre-casts R to fp8 e3m4 (values in [0,1) fit e3m4 with ~1% rel
    error; embeddings/stationaries stay bf16 so the PE contraction keeps
    bf16-grade accuracy on the operand that matters most).
  - Row-shard across 8 cores (512 users each). Each core loads its shard
    twice, both in host-prepacked fully-contiguous-per-partition layouts:
      At [128, NI*ULOC] fp8  (transposed: item-partition, user-free)
      Rn [128, NU*N]   fp8  (natural: user-partition, item-free)
  - PE matmuls with the big matrix as the moving operand (512-wide):
      C2:  u1_un.T = i0.T @ At        (32 MMs, accum over item chunks)
      C14: [z|w].T = [u0|u0+u1].T @ Rn (8 n-chunks x 4 user chunks)
      C3:  u2_un.T = i1.T @ At        (32 MMs)
  - deg_u / deg_i computed on host (exact, free); device gets reciprocals.
  - z = R^T u0 partials are AllReduced in PAIRS of behaviors (one
    [2,64,4096] bf16 collective per pair) to halve the serial ncfw time.
  - Item-side output assembled on host from per-core bf16 w partials.
  - All-ones matrices (virtual behaviors at init) detected on the host and
    computed analytically.

kernel(**inputs) takes the full unsharded inputs and returns [14, 4096, 64].
"""

import os
import numpy as np
import ml_dtypes

EPS = 1e-8
N, D = 4096, 64
P = 128
NCORES = 8
ULOC = N // NCORES          # 512 users per core
NU = ULOC // P              # 4 user chunks
NI = N // P                 # 32 item chunks
CH = 512                    # moving free-dim chunk
NCH = N // CH               # 8 chunks for the user-side contractions

_BF16 = ml_dtypes.bfloat16
_FP8 = ml_dtypes.float8_e3m4


# --------------------------------------------------------------------------
# device program
# --------------------------------------------------------------------------

def build_program(nb):
    """Build + bacc-compile the SPMD program for `nb` dense behaviors."""
    import concourse.bass as bass  # noqa: F401  (registers types)
    import concourse.mybir as mybir
    import concourse.tile as tile
    from concourse import bacc
    from concourse.masks import make_identity

    f32, bf16 = mybir.dt.float32, mybir.dt.bfloat16
    fp8 = mybir.dt.float8e3
    ALU = mybir.AluOpType
    CPY = mybir.ActivationFunctionType.Copy

    nc = bacc.Bacc("TRN2", target_bir_lowering=False, debug=False,
                   num_devices=NCORES)

    At_in = [nc.dram_tensor(f"At{b}", [P, NI * ULOC], fp8, kind="ExternalInput")
             for b in range(nb)]
    Rn_in = [nc.dram_tensor(f"Rn{b}", [P, NU * N], fp8, kind="ExternalInput")
             for b in range(nb)]
    ri_in = nc.dram_tensor("ri", [P, nb, NI, 1], f32, kind="ExternalInput")
    ru_in = nc.dram_tensor("ru", [P, nb, NU, 1], f32, kind="ExternalInput")
    i0s_in = nc.dram_tensor("i0s", [P, NI, D], bf16, kind="ExternalInput")
    u0s_in = nc.dram_tensor("u0s", [P, NU, D], bf16, kind="ExternalInput")
    uacc_out = [nc.dram_tensor(f"uacc{b}", [P, NU, D], f32, kind="ExternalOutput")
                for b in range(nb)]
    w_out = [nc.dram_tensor(f"wT{b}", [D, N], bf16, kind="ExternalOutput")
             for b in range(nb)]

    rg = [list(range(NCORES))]
    # asymmetric: the last AllReduce carries a single behavior so the
    # post-AR tail (its back phase) is as short as possible
    if 2 <= nb <= 4:
        groups = [list(range(nb - 1)), [nb - 1]]
    elif nb == 1:
        groups = [[0]]
    else:
        groups = [list(range(i, min(i + 2, nb))) for i in range(0, nb, 2)]
    defer_last = nb <= 4
    grp_of = {}
    for gi, g in enumerate(groups):
        for b in g:
            grp_of[b] = gi

    with tile.TileContext(nc) as tc:
        with (
            tc.tile_pool(name="big", bufs=min(nb, 4)) as pbig,
            tc.tile_pool(name="rn", bufs=2) as prn,
            tc.tile_pool(name="zw", bufs=2) as pzw,
            tc.tile_pool(name="small", bufs=2) as psm,
            # u1f crosses the front/back barrier (produced by front b,
            # read by back b): needs a buffer per behavior or the pool
            # rotation deadlocks against the barrier
            tc.tile_pool(name="u1f", bufs=min(nb, 4)) as pu1,
            tc.tile_pool(name="one", bufs=1) as pone,
            tc.tile_pool(name="uat", bufs=2) as pu,
            tc.tile_pool(name="mm", bufs=2, space="PSUM") as pmm,
            tc.tile_pool(name="mm14", bufs=3, space="PSUM") as pmm14,
            tc.tile_pool(name="tr", bufs=1, space="PSUM") as ptr,
            tc.tile_pool(name="dram", bufs=2, space="DRAM") as pdr,
        ):
            # tiny dummy collective issued first: it absorbs the runtime's
            # rank-sync barrier AND the ~12us first-collective ncfw setup,
            # so the real AllReduces dequeue with ~1us trigger latency
            dum = pone.tile([P, 8], f32)
            nc.vector.memset(dum[:], 0.0)
            dum_in = pdr.tile([P, 8], f32, tag="dumin", name="dumin")
            nc.sync.dma_start(out=dum_in[:], in_=dum[:])
            dum_out = pdr.tile([P, 8], f32, tag="dumout", name="dumout",
                               addr_space="Shared")
            nc.gpsimd.collective_compute(
                "AllReduce", ALU.add, replica_groups=rg,
                ins=[dum_in.opt()], outs=[dum_out.opt()])

            ident = pone.tile([P, P], f32)
            make_identity(nc, ident[:])
            i0s = pone.tile([P, NI, D], bf16)
            nc.sync.dma_start(out=i0s[:], in_=i0s_in[:])
            u0s = pone.tile([P, NU, D], bf16)
            nc.sync.dma_start(out=u0s[:], in_=u0s_in[:])
            ri_t = pone.tile([P, nb, NI, 1], f32)
            nc.sync.dma_start(out=ri_t[:], in_=ri_in[:])
            ru_t = pone.tile([P, nb, NU, 1], f32)
            nc.sync.dma_start(out=ru_t[:], in_=ru_in[:])

            at_tiles = {}
            state = {}
            zin_tiles = {}
            zout_tiles = {}

            def load_at(b):
                # host-prepacked: fully contiguous per partition; split in 2
                # so the first C2 matmuls can start at the halfway mark
                At = pbig.tile([P, NI, ULOC], fp8, tag="At", name=f"At{b}")
                src = At_in[b].ap().rearrange("p (ic u) -> p ic u", u=ULOC)
                h = NI // 2
                nc.scalar.dma_start(out=At[:, 0:h, :], in_=src[:, 0:h, :])
                nc.scalar.dma_start(out=At[:, h:NI, :], in_=src[:, h:NI, :])
                at_tiles[b] = At

            def load_rn(b):
                # Rn rides the SP ring, At the ACT ring: the two HWDGE
                # rings stream the two big operands in parallel
                Rn = prn.tile([P, NU, N], fp8, tag="Rn", name=f"Rn{b}")
                nc.sync.dma_start(
                    out=Rn[:], in_=Rn_in[b].ap().rearrange(
                        "p (uc n) -> p uc n", n=N))
                return Rn

            def front(b, Rn):
                At = at_tiles[b]
                gi = grp_of[b]
                g = groups[gi]

                # ---- C2: [64, 512] = i0.T @ At.  M=64 only fills half the
                # PE columns, so run two interleaved accumulation chains on
                # separate column groups (even item chunks -> psum rows
                # 0:64, odd -> 64:128); they execute concurrently.
                P2 = pmm.tile([P, CH], f32, tag="PC", name=f"P2_{b}")
                h = NI // 2
                for j in range(h):
                    nc.tensor.matmul(P2[0:D, :], i0s[:, 2 * j, :],
                                     At[:, 2 * j, :],
                                     start=(j == 0), stop=(j == h - 1))
                    nc.tensor.matmul(P2[D:2 * D, :], i0s[:, 2 * j + 1, :],
                                     At[:, 2 * j + 1, :],
                                     start=(j == 0), stop=(j == h - 1))
                S2 = psm.tile([D, CH], f32, tag="S2", name=f"S2_{b}")
                nc.vector.tensor_copy(out=S2[:], in_=P2[0:D, :])
                nc.vector.tensor_add(out=S2[:], in0=P2[D:2 * D, :], in1=S2[:])
                PT2 = ptr.tile([P, NU, D], f32, tag="PT2", name=f"PT2_{b}")
                for uc in range(NU):
                    nc.tensor.transpose(PT2[:, uc, :],
                                        S2[:, uc * P:(uc + 1) * P],
                                        ident[0:D, 0:D])
                # critical path to C14: L = [u0 | u0 + u1] in bf16
                L = psm.tile([P, NU, 2 * D], bf16, tag="L", name=f"L{b}")
                nc.vector.tensor_copy(out=L[:, :, 0:D], in_=u0s[:])
                for uc in range(NU):
                    nc.vector.scalar_tensor_tensor(
                        out=L[:, uc, D:2 * D], in0=PT2[:, uc, :],
                        scalar=ru_t[:, b, uc, :], in1=u0s[:, uc, :],
                        op0=ALU.mult, op1=ALU.add)
                # u1 (f32) for the layer average -- consumes PT2 right away
                # so its PSUM bank frees before the C14 stretch
                u1f = pu1.tile([P, NU, D], f32, tag="u1f", name=f"u1f{b}")
                rub = ru_t[:, b].broadcast_to([P, NU, D])
                nc.vector.tensor_mul(out=u1f[:], in0=PT2[:], in1=rub)

                # ---- C14: [z|w].T = [u0|u0+u1].T @ Rn, 8 chunks of 512
                zw = pzw.tile([P, N], bf16, tag="zw", name=f"zw{b}")
                for n in range(NCH):
                    P14 = pmm14.tile([P, CH], f32, tag="P14",
                                     name=f"P14_{b}_{n}")
                    for uc in range(NU):
                        nc.tensor.matmul(P14[:], L[:, uc, :],
                                         Rn[:, uc, n * CH:(n + 1) * CH],
                                         start=(uc == 0), stop=(uc == NU - 1))
                    nc.vector.tensor_copy(out=zw[:, n * CH:(n + 1) * CH],
                                          in_=P14[:])

                # ---- stores on the SP ring (w first, z last: the pair
                # AllReduce's semaphore wait then resolves to the z store
                # itself -- nothing later ever lands on this ring)
                if b == g[0]:
                    zin = pdr.tile([len(g), D, N], bf16, tag="zin",
                                   name=f"zin{gi}")
                    zin_tiles[gi] = zin
                zin = zin_tiles[gi]
                nc.sync.dma_start(out=w_out[b].ap(), in_=zw[D:2 * D, :])
                nc.sync.dma_start(out=zin[b - g[0]], in_=zw[0:D, :])
                state[b] = u1f

                # the LAST group's AllReduce is issued after the
                # front/back barrier (see below); earlier groups here
                if b == g[-1] and (gi < len(groups) - 1 or not defer_last):
                    issue_ar(gi)

            def issue_ar(gi):
                g = groups[gi]
                zout = pdr.tile([len(g), D, N], bf16, tag="zout",
                                name=f"zout{gi}", addr_space="Shared")
                zout_tiles[gi] = zout
                nc.gpsimd.collective_compute(
                    "AllReduce", ALU.add, replica_groups=rg,
                    ins=[zin_tiles[gi].opt()], outs=[zout.opt()])

            def back(b):
                At = at_tiles.pop(b)
                u1f = state.pop(b)
                gi = grp_of[b]
                zout = zout_tiles[gi]

                # ---- i1 = (z * ri) via xbar transpose + one broadcast mult
                # (back-chain DMAs ride the ACT ring, which is empty of
                # front work by now; the SP ring stays free of AR-waiting
                # entries so collective triggers never queue behind them)
                zs = psm.tile([D, N], bf16, tag="zs", name=f"zs{b}")
                nc.scalar.dma_start(out=zs[:], in_=zout[b - groups[gi][0]])
                i1b = psm.tile([P, NI, D], bf16, tag="i1b", name=f"i1b{b}")
                nc.scalar.dma_start_transpose(out=i1b[:], in_=zs[:])
                i1s = psm.tile([P, NI, D], bf16, tag="i1s", name=f"i1s{b}")
                rib = ri_t[:, b].broadcast_to([P, NI, D])
                nc.vector.tensor_mul(out=i1s[:], in0=i1b[:], in1=rib)

                # ---- C3: u2_un.T = i1.T @ At (same two-column-group trick)
                P3 = pmm.tile([P, CH], f32, tag="PC", name=f"P3_{b}")
                h = NI // 2
                for j in range(h):
                    nc.tensor.matmul(P3[0:D, :], i1s[:, 2 * j, :],
                                     At[:, 2 * j, :],
                                     start=(j == 0), stop=(j == h - 1))
                    nc.tensor.matmul(P3[D:2 * D, :], i1s[:, 2 * j + 1, :],
                                     At[:, 2 * j + 1, :],
                                     start=(j == 0), stop=(j == h - 1))
                S3 = psm.tile([D, CH], f32, tag="S3", name=f"S3_{b}")
                nc.vector.tensor_copy(out=S3[:], in_=P3[0:D, :])
                nc.vector.tensor_add(out=S3[:], in0=P3[D:2 * D, :], in1=S3[:])
                PT3 = ptr.tile([P, NU, D], f32, tag="PT3", name=f"PT3_{b}")
                for uc in range(NU):
                    nc.tensor.transpose(PT3[:, uc, :],
                                        S3[:, uc * P:(uc + 1) * P],
                                        ident[0:D, 0:D])
                uacc = pu.tile([P, NU, D], f32, tag="uacc", name=f"uacc{b}")
                for uc in range(NU):
                    nc.vector.scalar_tensor_tensor(
                        out=uacc[:, uc, :], in0=PT3[:, uc, :],
                        scalar=ru_t[:, b, uc, :], in1=u1f[:, uc, :],
                        op0=ALU.mult, op1=ALU.add)
                nc.scalar.dma_start(out=uacc_out[b].ap(), in_=uacc[:])

            if nb <= 4:
                # flat order: all fronts stream back-to-back (PE + DMA
                # dense), pair AllReduces overlap the following fronts,
                # backs run last (their AR waits are then mostly satisfied)
                load_at(0)
                rn = load_rn(0)
                for b in range(nb):
                    if b + 1 < nb:
                        load_at(b + 1)
                        rn_next = load_rn(b + 1)
                    else:
                        rn_next = None
                    front(b, rn)
                    rn = rn_next
                # Hard front/back fence.  HWDGE lane semaphores only count
                # completions, so a back-chain DMA linearized before a z
                # store silently pushes the AllReduce trigger thresholds
                # out to post-AR events (measured: 80us-late triggers).
                # The barrier is runtime-free here: backs are gated on the
                # first AllReduce anyway, which outlives every front op.
                # The last group's AllReduce is issued after it so the
                # barrier's own wait only covers the first one.
                tc.strict_bb_all_engine_barrier()
                issue_ar(len(groups) - 1)
                for b in range(nb):
                    back(b)
            else:
                # conservative interleave for unusual behavior counts
                load_at(0)
                rn = load_rn(0)
                for b in range(nb):
                    if b + 1 < nb:
                        load_at(b + 1)
                        rn_next = load_rn(b + 1)
                    else:
                        rn_next = None
                    front(b, rn)
                    rn = rn_next
                    if b >= 2:
                        back(b - 2)
                for b in range(max(nb - 2, 0), nb):
                    back(b)

    nc.compile()
    _fix_cc_waits(nc)
    return nc


def _fix_cc_waits(nc):
    """Rewrite each collective's semaphore wait to the exact completion
    values of its true DMA dependencies.

    The Tile scheduler lowers a collective's multi-dep wait to a single
    "ring clock" semaphore threshold chosen from its simulated
    linearization; when post-collective DMAs get linearized earlier, the
    chosen threshold is only reached by those later DMAs at runtime and the
    collective trigger stalls behind unrelated work (measured: 49us late).
    ge-imm waits are pure reads, so tightening them to the dependencies'
    own (sem, cumulative value) pairs is always sound.
    """
    import concourse.mybir as mybir

    f = nc.m.functions[0]
    insts = [i for b in f.blocks for i in b.instructions]
    cum = {}
    val_of = {}
    for i in insts:
        si = i.sync_info
        if not si:
            continue
        for u in (si.on_update or []):
            nm = u.ant_name or ''
            if 'DMAHW' in nm or 'DMASW' in nm:
                cum[nm] = cum.get(nm, 0) + u.update_value
                val_of[i.name] = (nm, u.id, cum[nm])
    # The wait pass can also emit standalone pre-wait events on the Pool
    # queue right before each collective (overflow waits beyond the
    # trigger's single slot).  Those carry the same mis-predicted ring
    # thresholds, so neutralize them; the exact wait below covers all deps.
    pool_prewaits = {}
    prev = None
    for i in insts:
        if str(getattr(i, 'engine', '')) != 'EngineType.Pool':
            continue
        if isinstance(i, mybir.InstCollectiveCompute):
            if (prev is not None
                    and isinstance(prev, mybir.InstEventSemaphore)
                    and prev.sync_info
                    and not (prev.sync_info.on_update or [])
                    and all('DMA' in (w.ant_name or '')
                            for w in (prev.sync_info.on_wait or []))):
                pool_prewaits[i.name] = prev
        prev = i
    for c in insts:
        if not isinstance(c, mybir.InstCollectiveCompute):
            continue
        waits, ok = [], True
        for dep, info in c.dependency_edges():
            if not info.sync:
                continue
            if dep in val_of:
                waits.append(val_of[dep])
            else:
                ok = False
        if ok and waits:
            # The collective trigger ucode supports a single sync wait.  All
            # deps here are z stores on the same SP HWDGE ring (issued and
            # drained FIFO), so the dep latest in the global round-robin
            # lane order dominates the others.  Lane index recovers the
            # global order: idx = (value/16 - 1)*8 + lane.
            def gidx(w):
                nm, _, v = w
                lane = int(nm.split('_')[0].replace('DMAHW', ''))
                return (v // 16 - 1) * 8 + lane
            nm, sid, v = max(waits, key=gidx)
            sw = [mybir.SyncWait(sync_type='semaphore', id=sid, ant_name=nm,
                                 wait_mode='sem-ge-imm', wait_value=v,
                                 wait_reg=None)]
            c.sync_info = mybir.SyncInfo(on_wait=sw,
                                         on_update=c.sync_info.on_update)
            pw = pool_prewaits.get(c.name)
            if pw is not None:
                pw.sync_info = mybir.SyncInfo(
                    on_wait=[], on_update=pw.sync_info.on_update)


# --------------------------------------------------------------------------
# host-side helpers
# --------------------------------------------------------------------------

def _swz_items(x):
    """[4096, C] -> [128, 32, C] with item = ic*128 + p."""
    return np.ascontiguousarray(x.reshape(NI, P, x.shape[1]).transpose(1, 0, 2))


def _swz_users(x):
    """[512, C] -> [128, 4, C] with user = uc*128 + p."""
    return np.ascontiguousarray(x.reshape(NU, P, x.shape[1]).transpose(1, 0, 2))


def host_prep_behavior(R):
    """fp8-cast + exact degree reciprocals (host, free)."""
    R8 = R.astype(_FP8)
    deg_i = R.sum(axis=0, dtype=np.float64)
    deg_u = R.sum(axis=1, dtype=np.float64)
    ri_vec = (1.0 / (deg_i + EPS)).astype(np.float32)
    # [P, NI, 1] with item = ic*128 + p
    ri_nat = np.ascontiguousarray(
        ri_vec.reshape(NI, P).T.reshape(P, NI, 1))
    return R8, ri_nat, deg_u, deg_i.astype(np.float32)


def prep_in_maps(prepped, u0, i0):
    """prepped: list of (R8, ri_nat, deg_u, deg_i) per dense behavior."""
    nb = len(prepped)
    i0s = _swz_items(i0.astype(_BF16))
    ri_all = np.ascontiguousarray(
        np.stack([p[1] for p in prepped], axis=1))        # [P, nb, NI, 1]
    in_maps = []
    for k in range(NCORES):
        m = {"i0s": i0s,
             "u0s": _swz_users(u0[k * ULOC:(k + 1) * ULOC].astype(_BF16)),
             "ri": ri_all}
        ru_l = []
        for b, (R8, ri_nat, deg_u, _) in enumerate(prepped):
            shard = R8[k * ULOC:(k + 1) * ULOC, :]
            m[f"At{b}"] = np.ascontiguousarray(
                shard.T.reshape(NI, P, ULOC).transpose(1, 0, 2)
                .reshape(P, NI * ULOC))
            m[f"Rn{b}"] = np.ascontiguousarray(
                shard.reshape(NU, P, N).transpose(1, 0, 2).reshape(P, NU * N))
            du = deg_u[k * ULOC:(k + 1) * ULOC]
            ru = (1.0 / (du + EPS)).astype(np.float32)
            ru_l.append(ru.reshape(NU, P).T.reshape(P, NU, 1))
        m["ru"] = np.ascontiguousarray(np.stack(ru_l, axis=1))  # [P,nb,NU,1]
        in_maps.append(m)
    return in_maps


def assemble_dense(results, degs, nb):
    """Per-behavior (u_acc [N,D], i_acc [N,D]) from per-core outputs."""
    out = []
    for b in range(nb):
        u = np.concatenate(
            [results[k][f"uacc{b}"].transpose(1, 0, 2).reshape(ULOC, D)
             for k in range(NCORES)], axis=0) * np.float32(0.5)
        w = np.sum([results[k][f"wT{b}"].astype(np.float32)
                    for k in range(NCORES)], axis=0, dtype=np.float32)
        i_acc = (w * np.float32(0.5)
                 / (degs[b] + np.float32(EPS))[None, :]).T
        out.append((np.ascontiguousarray(u, dtype=np.float32),
                    np.ascontiguousarray(i_acc, dtype=np.float32)))
    return out


def ones_behavior(u0, i0):
    """Analytic LightGCN-2-layer outputs when R is all-ones [N, N]."""
    s_i = i0.astype(np.float64).sum(axis=0)
    s_u = u0.astype(np.float64).sum(axis=0)
    d = N + EPS
    u_row = (s_i / d + s_u * N / (d * d)) * 0.5
    i_row = (s_u / d + s_i * N / (d * d)) * 0.5
    u = np.broadcast_to(u_row.astype(np.float32), (N, D)).copy()
    it = np.broadcast_to(i_row.astype(np.float32), (N, D)).copy()
    return u, it


# --------------------------------------------------------------------------
# cached device runner (compile once per behavior-count, run many)
# --------------------------------------------------------------------------

_RUNNERS = {}


class _Runner:
    def __init__(self, nb):
        self.nb = nb
        self.nc = build_program(nb)
        self._jitted = None
        self._meta = None

    def _prep_jit(self):
        import jax
        import numpy as _np
        from jax.sharding import Mesh, PartitionSpec
        from jax.experimental.shard_map import shard_map
        from concourse import bass2jax
        from concourse.bass2jax import _bass_exec_p, partition_id_tensor
        import concourse.mybir as mybir

        bass2jax.install_neuronx_cc_hook()
        nc = self.nc
        partition_name = (nc.partition_id_tensor.name
                          if nc.partition_id_tensor else None)
        in_names, out_names, out_avals, zero_shapes = [], [], [], []
        for alloc in nc.m.functions[0].allocations:
            if not isinstance(alloc, mybir.MemoryLocationSet):
                continue
            name = alloc.memorylocations[0].name
            if alloc.kind == "ExternalInput":
                if name != partition_name:
                    in_names.append(name)
            elif alloc.kind == "ExternalOutput":
                shape = tuple(alloc.tensor_shape)
                dtype = mybir.dt.np(alloc.dtype)
                out_names.append(name)
                out_avals.append(jax.core.ShapedArray(shape, dtype))
                zero_shapes.append((shape, dtype))
        n_params = len(in_names)
        full_in_names = list(in_names) + list(out_names)
        if partition_name is not None:
            full_in_names.append(partition_name)

        def _body(*args):
            operands = list(args)
            if partition_name is not None:
                operands.append(partition_id_tensor())
            outs = _bass_exec_p.bind(
                *operands,
                out_avals=tuple(out_avals),
                in_names=tuple(full_in_names),
                out_names=tuple(out_names),
                lowering_input_output_aliases=(),
                sim_require_finite=True,
                sim_require_nnan=True,
                nc=nc,
            )
            return tuple(outs)

        devices = jax.devices()[:NCORES]
        mesh = Mesh(_np.asarray(devices), ("core",))
        n_outs = len(out_names)
        in_specs = (PartitionSpec("core"),) * (n_params + n_outs)
        out_specs = (PartitionSpec("core"),) * n_outs
        donate = tuple(range(n_params, n_params + n_outs))
        self._jitted = jax.jit(
            shard_map(_body, mesh=mesh, in_specs=in_specs,
                      out_specs=out_specs, check_rep=False),
            donate_argnums=donate, keep_unused=True)
        self._meta = (in_names, out_names, out_avals, zero_shapes, n_params)

    def run(self, in_maps):
        if self._jitted is None:
            self._prep_jit()
        import numpy as _np
        in_names, out_names, out_avals, zero_shapes, n_params = self._meta
        concat_in = [
            _np.concatenate([_np.asarray(in_maps[c][nm]) for c in range(NCORES)],
                            axis=0)
            for nm in in_names]
        concat_zeros = [_np.zeros((NCORES * s[0], *s[1:]), dt)
                        for (s, dt) in zero_shapes]
        out_arrs = self._jitted(*concat_in, *concat_zeros)
        results = []
        for c in range(NCORES):
            results.append({
                nm: _np.asarray(out_arrs[i]).reshape(
                    NCORES, *out_avals[i].shape)[c]
                for i, nm in enumerate(out_names)})
        return results

    def run_traced(self, in_maps, tmpdir=None):
        """Run through run_bass_kernel_spmd with NTFF tracing (recompiles)."""
        _install_trace_shims()
        from concourse.bass_utils import run_bass_kernel_spmd
        return run_bass_kernel_spmd(self.nc, in_maps,
                                    core_ids=list(range(NCORES)),
                                    trace=True, tmpdir=tmpdir)


def _install_trace_shims():
    """This image's antenv lacks axon_hooks (the NTFF-hook registry) and has
    no artifact bucket; recreate the hook from the boot recipe and make
    artifact upload a local no-op."""
    import sys, types, importlib.util

    if "antenv.axon_hooks" not in sys.modules:
        mod = types.ModuleType("antenv.axon_hooks")
        mod._hook = None

        def set_axon_ntff_profile_hook(h):
            mod._hook = h

        def get_axon_ntff_profile_hook():
            return mod._hook

        mod.set_axon_ntff_profile_hook = set_axon_ntff_profile_hook
        mod.get_axon_ntff_profile_hook = get_axon_ntff_profile_hook
        import antenv
        sys.modules["antenv.axon_hooks"] = mod
        antenv.axon_hooks = mod

        spec = importlib.util.spec_from_file_location(
            "trn_boot_shim", "/root/.axon_site/trn_agent_boot/trn_boot.py")
        boot = importlib.util.module_from_spec(spec)
        spec.loader.exec_module(boot)
        hook = boot._ntff_profile_via_ctypes("/opt/axon/libaxon_pjrt.so")
        mod._hook = hook

    import concourse.bass_utils as bu
    if not getattr(bu.upload_artifacts, "_is_local_shim", False):
        def _local_upload(tmpdir):
            return tmpdir
        _local_upload._is_local_shim = True
        bu.upload_artifacts = _local_upload


def get_runner(nb):
    if nb not in _RUNNERS:
        _RUNNERS[nb] = _Runner(nb)
    return _RUNNERS[nb]


# --------------------------------------------------------------------------
# entry point
# --------------------------------------------------------------------------

def _is_ones(a):
    return a[0, 0] == 1.0 and bool(np.all(a == np.float32(1.0)))


def kernel(**inputs):
    inputs = {k: np.asarray(v) for k, v in inputs.items()}
    u0 = np.ascontiguousarray(inputs["user_embedding"], dtype=np.float32)
    i0 = np.ascontiguousarray(inputs["item_embedding"], dtype=np.float32)

    real_names = ["R_click", "R_fav", "R_cart", "R_buy"]
    virt_names = [("M_click", "add_click"), ("M_fav", "add_fav"),
                  ("M_cart", "add_cart")]
    mats = [np.asarray(inputs[n], dtype=np.float32) for n in real_names]
    mats += [np.asarray(inputs[m], dtype=np.float32) for m, _ in virt_names]

    dense_idx = [j for j, a in enumerate(mats) if not _is_ones(a)]
    per_behavior = [None] * 7

    if dense_idx:
        nb = len(dense_idx)
        runner = get_runner(nb)
        prepped = [host_prep_behavior(mats[j]) for j in dense_idx]
        in_maps = prep_in_maps(prepped, u0, i0)
        results = runner.run(in_maps)
        dense = assemble_dense(results, [p[3] for p in prepped], nb)
        for pos, j in enumerate(dense_idx):
            per_behavior[j] = dense[pos]

    ones_cache = None
    for j, a in enumerate(mats):
        if per_behavior[j] is None:
            if ones_cache is None:
                ones_cache = ones_behavior(u0, i0)
            per_behavior[j] = ones_cache

    ur = [per_behavior[j][0] for j in range(4)]
    ir = [per_behavior[j][1] for j in range(4)]
    uv = [per_behavior[4 + j][0] + np.asarray(inputs[virt_names[j][1]],
                                              dtype=np.float32)
          for j in range(3)]
    iv = [per_behavior[4 + j][1] for j in range(3)]

    out = np.concatenate(
        [np.stack(ur), np.stack(ir), np.stack(uv), np.stack(iv)], axis=0)
    return np.ascontiguousarray(out, dtype=np.float32)
